# revision 3
# baseline (speedup 1.0000x reference)
"""Trainium2 Bass kernel for nn_EntropyController.

Math (per batch b of features F in R^{N x D}):
    d2_ij   = max(|F_i|^2 + |F_j|^2 - 2 F_i.F_j, 0)
    K_ij    = exp(-d2_ij / (2 tau^2))
    P_ij    = K_ij / S_i,  S_i = sum_j K_ij
    ent_i   = -sum_j P_ij log(P_ij + 1e-6)
    ctrl_i  = sigmoid(-(ent_i - target)/tau)
    out_i   = F_i * ctrl_i            (plus ctrl itself as a second output)

Device formulation: let t_ij = F_i.F_j/tau^2 + a_i + a_j with
a_i = -|F_i|^2/(2 tau^2), i.e. t_ij = -d2_ij/(2 tau^2) (no clamp; t <= ~0 up
to rounding and exp is computed directly from it). Then with U_i =
sum_j t_ij K_ij:
    ent_i = log(S_i) - U_i / S_i
which drops the 1e-6 inside the log; the resulting entropy error is bounded
by N*1e-6 ~ 4e-3 absolute in the worst case, and after the sigmoid (slope
<= 1/4) it is far below fp32 comparison tolerances for these magnitudes.

Sharding: batch b -> core pair (2b, 2b+1); each core handles 2048 query rows
against all 4096 keys of its batch.  Per core the gram tile [128, 4096] is
produced by the tensor engine from a host-pretransposed F^T (fp32r matmuls,
contraction K=512 as 4x128 + a K=1 augmentation matmul that adds a_j), the
exp + row-sum S runs on the scalar engine (Exp activation with per-partition
bias a_i and the free accumulator), and U comes from one fused DVE op
(affine_mul_reduce: (t + a_i) * K, row-accumulated).
"""

import sys

for _p in ("/opt/trn_rl_repo", "/root/.axon_site/_ro/trn_rl_repo"):
    if _p not in sys.path:
        sys.path.append(_p)

import numpy as np

from concourse import bacc, mybir
from concourse.tile import TileContext
from concourse.bass_utils import run_bass_kernel_spmd

F32 = mybir.dt.float32
F32R = mybir.dt.float32r
AF = mybir.ActivationFunctionType

B, N, D = 4, 4096, 512
NCORES = 8
HALF = N // 2            # query rows per core
RT = HALF // 128         # 16 row tiles per core
GROUP = 2048             # PSUM group (4 banks); 2 groups span the 4096 keys
NGROUPS = N // GROUP
CHUNK = 512              # single-matmul moving free dim (1 PSUM bank)
KC = D // 128            # 4 contraction chunks

_prog_cache = {}


def _build_program(tau: float, target: float):
    nc = bacc.Bacc("TRN2", target_bir_lowering=False, debug=False,
                   num_devices=NCORES)
    ft = nc.dram_tensor("ft", [D, N], F32R, kind="ExternalInput")
    arow = nc.dram_tensor("arow", [1, N], F32R, kind="ExternalInput")
    acol = nc.dram_tensor("acol", [128, RT], F32, kind="ExternalInput")
    ones = nc.dram_tensor("ones", [1, 128], F32R, kind="ExternalInput")
    fnat = nc.dram_tensor("fnat", [HALF, D], F32, kind="ExternalInput")
    outf = nc.dram_tensor("outf", [HALF, D], F32, kind="ExternalOutput")
    ctrl = nc.dram_tensor("ctrl", [128, RT], F32, kind="ExternalOutput")

    fnat_r = fnat.rearrange("(s p) d -> p s d", p=128)   # [128, RT, D]
    outf_r = outf.rearrange("(s p) d -> p s d", p=128)

    with TileContext(nc) as tc:
        with (
            tc.tile_pool(name="singles", bufs=1) as singles,
            tc.tile_pool(name="psum", bufs=2, space="PSUM") as psumz,
            tc.tile_pool(name="kpool", bufs=2) as kpool,
            tc.tile_pool(name="wpool", bufs=2) as wpool,
            tc.tile_pool(name="fpool", bufs=2) as fpool,
            tc.tile_pool(name="opool", bufs=2) as opool,
            tc.tile_pool(name="parts", bufs=3) as parts,
            tc.tile_pool(name="sm", bufs=12) as sm,
        ):
            ft_sb = [
                singles.tile([128, N], F32R, name=f"ftsb{i}", tag=f"ftsb{i}")
                for i in range(KC)
            ]
            for i in range(KC):
                nc.sync.dma_start(out=ft_sb[i][:], in_=ft[i * 128:(i + 1) * 128, :])
            arow_sb = singles.tile([1, N], F32R)
            nc.sync.dma_start(out=arow_sb[:], in_=arow[:])
            acol_sb = singles.tile([128, RT], F32)
            nc.sync.dma_start(out=acol_sb[:], in_=acol[:])
            ones_sb = singles.tile([1, 128], F32R)
            nc.sync.dma_start(out=ones_sb[:], in_=ones[:])
            ctrl_sb = singles.tile([128, RT], F32)
            ntgt_sb = singles.tile([128, 1], F32)
            nc.vector.memset(ntgt_sb[:], -target if tau == 1.0 else -target / tau)

            fnat_t = None
            out_t = None
            for r in range(RT):
                if r % 4 == 0:
                    fnat_t = fpool.tile([128, 4, D], F32, name="fnt", tag="fnt")
                    nc.sync.dma_start(out=fnat_t[:], in_=fnat_r[:, r:r + 4, :])
                    out_t = opool.tile([128, 4, D], F32, name="ot", tag="ot")
                a_r = acol_sb[:, r:r + 1]
                s_parts = parts.tile([128, NGROUPS], F32, name="sp", tag="sp")
                u_parts = parts.tile([128, NGROUPS], F32, name="up", tag="up")
                for g in range(NGROUPS):
                    psum = psumz.tile([128, GROUP], F32, name="ps", tag="ps")
                    for n in range(GROUP // CHUNK):
                        sl = slice(n * CHUNK, (n + 1) * CHUNK)
                        c0 = g * GROUP + n * CHUNK
                        for kc in range(KC):
                            nc.tensor.matmul(
                                psum[:, sl],
                                ft_sb[kc][:, r * 128:(r + 1) * 128],
                                ft_sb[kc][:, c0:c0 + CHUNK],
                                start=(kc == 0), stop=False,
                            )
                        nc.tensor.matmul(
                            psum[:, sl], ones_sb[:], arow_sb[:, c0:c0 + CHUNK],
                            start=False, stop=True,
                        )
                    k_t = kpool.tile([128, GROUP], F32, name="kt", tag="kt")
                    nc.scalar.activation(
                        out=k_t[:], in_=psum[:], func=AF.Exp,
                        bias=a_r, scale=1.0,
                        accum_out=s_parts[:, g:g + 1],
                    )
                    w_t = wpool.tile([128, GROUP], F32, name="wt", tag="wt")
                    nc.vector.affine_mul_reduce(
                        out=w_t[:], accum_out=u_parts[:, g:g + 1],
                        in0=psum[:], in1=k_t[:], scale=1.0, bias=a_r,
                    )
                # ---- per-row tail: ctrl = sigmoid(-(ent - target)/tau) ----
                s_row = sm.tile([128, 1], F32, name="srow", tag="srow")
                nc.vector.tensor_reduce(
                    s_row[:], s_parts[:], axis=mybir.AxisListType.X,
                    op=mybir.AluOpType.add,
                )
                u_row = sm.tile([128, 1], F32, name="urow", tag="urow")
                nc.vector.tensor_reduce(
                    u_row[:], u_parts[:], axis=mybir.AxisListType.X,
                    op=mybir.AluOpType.add,
                )
                r_s = sm.tile([128, 1], F32, name="rs", tag="rs")
                nc.vector.reciprocal(r_s[:], s_row[:])
                uos = sm.tile([128, 1], F32, name="uos", tag="uos")
                nc.vector.tensor_mul(uos[:], u_row[:], r_s[:])
                e2 = sm.tile([128, 1], F32, name="e2", tag="e2")
                if tau == 1.0:
                    # den = 1 + S * exp(-U/S - target)  (log-free; exp table only)
                    nc.scalar.activation(out=e2[:], in_=uos[:], func=AF.Exp,
                                         scale=-1.0, bias=ntgt_sb[:])
                    se = sm.tile([128, 1], F32, name="se", tag="se")
                    nc.vector.tensor_mul(se[:], s_row[:], e2[:])
                else:
                    # den = 1 + exp((lnS - U/S - target)/tau)
                    ln_s = sm.tile([128, 1], F32, name="lns", tag="lns")
                    nc.scalar.activation(out=ln_s[:], in_=s_row[:], func=AF.Ln)
                    ent = sm.tile([128, 1], F32, name="ent", tag="ent")
                    nc.vector.tensor_sub(ent[:], ln_s[:], uos[:])
                    se = sm.tile([128, 1], F32, name="se", tag="se")
                    nc.scalar.activation(out=se[:], in_=ent[:], func=AF.Exp,
                                         scale=1.0 / tau, bias=ntgt_sb[:])
                den = sm.tile([128, 1], F32, name="den", tag="den")
                nc.vector.tensor_scalar_add(den[:], se[:], 1.0)
                nc.vector.reciprocal(ctrl_sb[:, r:r + 1], den[:])
                nc.vector.tensor_scalar_mul(
                    out_t[:, r % 4, :], fnat_t[:, r % 4, :],
                    ctrl_sb[:, r:r + 1],
                )
                if r % 4 == 3:
                    nc.sync.dma_start(out=outf_r[:, r - 3:r + 1, :], in_=out_t[:])
            nc.sync.dma_start(out=ctrl[:], in_=ctrl_sb[:])
    nc.compile()
    return nc


def get_program(tau: float, target: float):
    key = (tau, target)
    if key not in _prog_cache:
        _prog_cache[key] = _build_program(tau, target)
    return _prog_cache[key]


def make_in_maps(features: np.ndarray, tau: float):
    ones = np.ones((1, 128), np.float32)
    in_maps = []
    for c in range(NCORES):
        b, h = divmod(c, 2)
        F = features[b]
        if h == 0:
            Fro = F
        else:
            Fro = np.concatenate([F[HALF:], F[:HALF]], axis=0)
        ft = np.ascontiguousarray(Fro.T.astype(np.float32) / np.float32(tau))
        a = (-(Fro.astype(np.float64) ** 2).sum(-1)
             / (2.0 * tau * tau)).astype(np.float32)
        in_maps.append({
            "ft": ft,
            "arow": np.ascontiguousarray(a[None, :]),
            "acol": np.ascontiguousarray(a[:HALF].reshape(RT, 128).T),
            "ones": ones,
            "fnat": np.ascontiguousarray(Fro[:HALF].astype(np.float32)),
        })
    return in_maps


def assemble(results):
    controlled = np.empty((B, N, D), np.float32)
    control = np.empty((B, N), np.float32)
    for c, r in enumerate(results):
        b, h = divmod(c, 2)
        controlled[b, h * HALF:(h + 1) * HALF] = r["outf"]
        control[b, h * HALF:(h + 1) * HALF] = r["ctrl"].T.reshape(HALF)
    return controlled, control


def kernel(features, W1, b1, ln_g, ln_b, W2, b2, target_entropy, temperature):
    features = np.asarray(features, dtype=np.float32)
    assert features.shape == (B, N, D), features.shape
    tau = float(np.asarray(temperature).reshape(-1)[0])
    target = float(np.asarray(target_entropy).reshape(-1)[0])
    nc = get_program(tau, target)
    in_maps = make_in_maps(features, tau)
    res = run_bass_kernel_spmd(nc, in_maps, core_ids=list(range(NCORES)))
    return assemble(res.results)


# revision 4
# speedup vs baseline: 1.1635x; 1.1635x over previous
"""Trainium2 Bass kernel for nn_EntropyController.

Math (per batch b of features F in R^{N x D}):
    d2_ij   = max(|F_i|^2 + |F_j|^2 - 2 F_i.F_j, 0)
    K_ij    = exp(-d2_ij / (2 tau^2))
    P_ij    = K_ij / S_i,  S_i = sum_j K_ij
    ent_i   = -sum_j P_ij log(P_ij + 1e-6)
    ctrl_i  = sigmoid(-(ent_i - target)/tau)
    out_i   = F_i * ctrl_i            (plus ctrl itself as a second output)

Device formulation: let t_ij = F_i.F_j/tau^2 + a_i + a_j with
a_i = -|F_i|^2/(2 tau^2), i.e. t_ij = -d2_ij/(2 tau^2) (no clamp; t <= ~0 up
to rounding and exp is computed directly from it). Then with U_i =
sum_j t_ij K_ij:
    ent_i = log(S_i) - U_i / S_i
which drops the 1e-6 inside the log; the resulting entropy error is bounded
by N*1e-6 ~ 4e-3 absolute in the worst case, and after the sigmoid (slope
<= 1/4) it is far below fp32 comparison tolerances for these magnitudes.

Sharding: batch b -> core pair (2b, 2b+1); each core handles 2048 query rows
against all 4096 keys of its batch.  Per core the gram tile [128, 4096] is
produced by the tensor engine from a host-pretransposed F^T (fp32r matmuls,
contraction K=512 as 4x128 + a K=1 augmentation matmul that adds a_j), the
exp + row-sum S runs on the scalar engine (Exp activation with per-partition
bias a_i and the free accumulator), and U comes from one fused DVE op
(affine_mul_reduce: (t + a_i) * K, row-accumulated).
"""

import sys

for _p in ("/opt/trn_rl_repo", "/root/.axon_site/_ro/trn_rl_repo"):
    if _p not in sys.path:
        sys.path.append(_p)

import numpy as np

from concourse import bacc, mybir
from concourse.tile import TileContext
from concourse.bass_utils import run_bass_kernel_spmd

F32 = mybir.dt.float32
F32R = mybir.dt.float32r
AF = mybir.ActivationFunctionType

B, N, D = 4, 4096, 512
NCORES = 8
HALF = N // 2            # query rows per core
RT = HALF // 128         # 16 row tiles per core
GROUP = 1024             # PSUM group (2 banks); 4 groups span the 4096 keys
NGROUPS = N // GROUP
CHUNK = 512              # single-matmul moving free dim (1 PSUM bank)
KC = D // 128            # 4 contraction chunks

_prog_cache = {}


def _build_program(tau: float, target: float):
    nc = bacc.Bacc("TRN2", target_bir_lowering=False, debug=False,
                   num_devices=NCORES)
    ft = nc.dram_tensor("ft", [D, N], F32R, kind="ExternalInput")
    arow = nc.dram_tensor("arow", [1, N], F32R, kind="ExternalInput")
    acol = nc.dram_tensor("acol", [128, RT], F32, kind="ExternalInput")
    ones = nc.dram_tensor("ones", [1, 128], F32R, kind="ExternalInput")
    fnat = nc.dram_tensor("fnat", [HALF, D], F32, kind="ExternalInput")
    outf = nc.dram_tensor("outf", [HALF, D], F32, kind="ExternalOutput")
    ctrl = nc.dram_tensor("ctrl", [128, RT], F32, kind="ExternalOutput")

    fnat_r = fnat.rearrange("(s p) d -> p s d", p=128)   # [128, RT, D]
    outf_r = outf.rearrange("(s p) d -> p s d", p=128)

    with TileContext(nc) as tc:
        with (
            tc.tile_pool(name="singles", bufs=1) as singles,
            tc.tile_pool(name="psum", bufs=4, space="PSUM") as psumz,
            tc.tile_pool(name="kpool", bufs=4) as kpool,
            tc.tile_pool(name="wpool", bufs=3) as wpool,
            tc.tile_pool(name="fpool", bufs=2) as fpool,
            tc.tile_pool(name="opool", bufs=2) as opool,
            tc.tile_pool(name="parts", bufs=3) as parts,
            tc.tile_pool(name="sm", bufs=12) as sm,
        ):
            ft_sb = [
                singles.tile([128, N], F32R, name=f"ftsb{i}", tag=f"ftsb{i}")
                for i in range(KC)
            ]
            # split the F^T loads into column pieces, issued in consumption
            # order, so the first matmuls don't wait for the full 2MB tile
            FT_PIECE = 1024
            for j in range(N // FT_PIECE):
                for i in range(KC):
                    nc.sync.dma_start(
                        out=ft_sb[i][:, j * FT_PIECE:(j + 1) * FT_PIECE],
                        in_=ft[i * 128:(i + 1) * 128, j * FT_PIECE:(j + 1) * FT_PIECE],
                    )
            arow_sb = singles.tile([1, N], F32R)
            nc.sync.dma_start(out=arow_sb[:], in_=arow[:])
            acol_sb = singles.tile([128, RT], F32)
            nc.sync.dma_start(out=acol_sb[:], in_=acol[:])
            ones_sb = singles.tile([1, 128], F32R)
            nc.sync.dma_start(out=ones_sb[:], in_=ones[:])
            ctrl_sb = singles.tile([128, RT], F32)
            ntgt_sb = singles.tile([128, 1], F32)
            nc.vector.memset(ntgt_sb[:], -target if tau == 1.0 else -target / tau)

            fnat_t = None
            out_t = None
            for r in range(RT):
                if r % 4 == 0:
                    fnat_t = fpool.tile([128, 4, D], F32, name="fnt", tag="fnt")
                    nc.sync.dma_start(out=fnat_t[:], in_=fnat_r[:, r:r + 4, :])
                    out_t = opool.tile([128, 4, D], F32, name="ot", tag="ot")
                a_r = acol_sb[:, r:r + 1]
                s_parts = parts.tile([128, NGROUPS], F32, name="sp", tag="sp")
                u_parts = parts.tile([128, NGROUPS], F32, name="up", tag="up")
                for g in range(NGROUPS):
                    psum = psumz.tile([128, GROUP], F32, name="ps", tag="ps")
                    for n in range(GROUP // CHUNK):
                        sl = slice(n * CHUNK, (n + 1) * CHUNK)
                        c0 = g * GROUP + n * CHUNK
                        for kc in range(KC):
                            nc.tensor.matmul(
                                psum[:, sl],
                                ft_sb[kc][:, r * 128:(r + 1) * 128],
                                ft_sb[kc][:, c0:c0 + CHUNK],
                                start=(kc == 0), stop=False,
                            )
                        nc.tensor.matmul(
                            psum[:, sl], ones_sb[:], arow_sb[:, c0:c0 + CHUNK],
                            start=False, stop=True,
                        )
                    k_t = kpool.tile([128, GROUP], F32, name="kt", tag="kt")
                    nc.scalar.activation(
                        out=k_t[:], in_=psum[:], func=AF.Exp,
                        bias=a_r, scale=1.0,
                        accum_out=s_parts[:, g:g + 1],
                    )
                    w_t = wpool.tile([128, GROUP], F32, name="wt", tag="wt")
                    nc.vector.affine_mul_reduce(
                        out=w_t[:], accum_out=u_parts[:, g:g + 1],
                        in0=psum[:], in1=k_t[:], scale=1.0, bias=a_r,
                    )
                # ---- per-row tail: ctrl = sigmoid(-(ent - target)/tau) ----
                s_row = sm.tile([128, 1], F32, name="srow", tag="srow")
                nc.vector.tensor_reduce(
                    s_row[:], s_parts[:], axis=mybir.AxisListType.X,
                    op=mybir.AluOpType.add,
                )
                u_row = sm.tile([128, 1], F32, name="urow", tag="urow")
                nc.vector.tensor_reduce(
                    u_row[:], u_parts[:], axis=mybir.AxisListType.X,
                    op=mybir.AluOpType.add,
                )
                r_s = sm.tile([128, 1], F32, name="rs", tag="rs")
                nc.vector.reciprocal(r_s[:], s_row[:])
                uos = sm.tile([128, 1], F32, name="uos", tag="uos")
                nc.vector.tensor_mul(uos[:], u_row[:], r_s[:])
                e2 = sm.tile([128, 1], F32, name="e2", tag="e2")
                if tau == 1.0:
                    # den = 1 + S * exp(-U/S - target)  (log-free; exp table only)
                    nc.scalar.activation(out=e2[:], in_=uos[:], func=AF.Exp,
                                         scale=-1.0, bias=ntgt_sb[:])
                    se = sm.tile([128, 1], F32, name="se", tag="se")
                    nc.vector.tensor_mul(se[:], s_row[:], e2[:])
                else:
                    # den = 1 + exp((lnS - U/S - target)/tau)
                    ln_s = sm.tile([128, 1], F32, name="lns", tag="lns")
                    nc.scalar.activation(out=ln_s[:], in_=s_row[:], func=AF.Ln)
                    ent = sm.tile([128, 1], F32, name="ent", tag="ent")
                    nc.vector.tensor_sub(ent[:], ln_s[:], uos[:])
                    se = sm.tile([128, 1], F32, name="se", tag="se")
                    nc.scalar.activation(out=se[:], in_=ent[:], func=AF.Exp,
                                         scale=1.0 / tau, bias=ntgt_sb[:])
                den = sm.tile([128, 1], F32, name="den", tag="den")
                nc.vector.tensor_scalar_add(den[:], se[:], 1.0)
                nc.vector.reciprocal(ctrl_sb[:, r:r + 1], den[:])
                nc.vector.tensor_scalar_mul(
                    out_t[:, r % 4, :], fnat_t[:, r % 4, :],
                    ctrl_sb[:, r:r + 1],
                )
                if r % 4 == 3:
                    nc.sync.dma_start(out=outf_r[:, r - 3:r + 1, :], in_=out_t[:])
            nc.sync.dma_start(out=ctrl[:], in_=ctrl_sb[:])
    nc.compile()
    return nc


def get_program(tau: float, target: float):
    key = (tau, target)
    if key not in _prog_cache:
        _prog_cache[key] = _build_program(tau, target)
    return _prog_cache[key]


def make_in_maps(features: np.ndarray, tau: float):
    ones = np.ones((1, 128), np.float32)
    in_maps = []
    for c in range(NCORES):
        b, h = divmod(c, 2)
        F = features[b]
        if h == 0:
            Fro = F
        else:
            Fro = np.concatenate([F[HALF:], F[:HALF]], axis=0)
        ft = np.ascontiguousarray(Fro.T.astype(np.float32) / np.float32(tau))
        a = (-(Fro.astype(np.float64) ** 2).sum(-1)
             / (2.0 * tau * tau)).astype(np.float32)
        in_maps.append({
            "ft": ft,
            "arow": np.ascontiguousarray(a[None, :]),
            "acol": np.ascontiguousarray(a[:HALF].reshape(RT, 128).T),
            "ones": ones,
            "fnat": np.ascontiguousarray(Fro[:HALF].astype(np.float32)),
        })
    return in_maps


def assemble(results):
    controlled = np.empty((B, N, D), np.float32)
    control = np.empty((B, N), np.float32)
    for c, r in enumerate(results):
        b, h = divmod(c, 2)
        controlled[b, h * HALF:(h + 1) * HALF] = r["outf"]
        control[b, h * HALF:(h + 1) * HALF] = r["ctrl"].T.reshape(HALF)
    return controlled, control


def kernel(features, W1, b1, ln_g, ln_b, W2, b2, target_entropy, temperature):
    features = np.asarray(features, dtype=np.float32)
    assert features.shape == (B, N, D), features.shape
    tau = float(np.asarray(temperature).reshape(-1)[0])
    target = float(np.asarray(target_entropy).reshape(-1)[0])
    nc = get_program(tau, target)
    in_maps = make_in_maps(features, tau)
    res = run_bass_kernel_spmd(nc, in_maps, core_ids=list(range(NCORES)))
    return assemble(res.results)
